# revision 12
# baseline (speedup 1.0000x reference)
"""Trainium2 Bass kernel for a 2-layer GCN fingerprint network.

    h   = relu(x @ W_i + b_i)                  [N, 128] -> [N, 64]
    z   = gcn_conv(h, edge_index, W_c)         scatter/gather over E edges
    h2  = relu(z @ W_h + b_h)
    out = h2 @ W_o + b_o                       [N, 1]

Strategy (8 NeuronCores, full input in / full output out):
  - The per-edge norm factors into per-node scales: with dis = outdeg^-0.5,
      y   = dis * ((relu(x@W_i+b_i)) @ W_c)          (per-node)
      z_d = dis_d * sum_{e: col(e)=d} y[row(e)]      (gather + segment sum)
  - Phase A (replicated on every core): compute the full y table [NPAD, 64]
    fp32 into DRAM.  x is host-pretransposed to bf16 [128, NPAD].
  - Phase B (dst-sharded): edges sorted by destination; destinations sorted
    by (lo-degree, hi-degree) and grouped into 128-dst blocks.  Per block,
    dma_gather (int16 indices, so the y-table is addressed as a <32768-row
    "lo" part and the rest as "hi") lands edge e's 256B y-row at
    [partition = dst_rel, slot]; a halving-tree add on the vector engine
    reduces the slots; a small matmul tail (transpose, W_h, relu, W_o)
    finishes each block.
  - Pad nodes sit at BOTH ends of the column space so both the lo and hi
    table ranges contain all-zero rows for slot padding.
  - Block slot-count schedules are compile-time constants shared by all
    cores (SPMD): global blocks are dealt round-robin; K_j = max over the 8
    blocks dealt at step j (tight because blocks are degree-sorted).

The graph structure (edge_index) is known when kernel() is called, so all
index/slot layout is precomputed on the host; the device only moves floats.
"""

import sys

sys.path.insert(0, "/opt/trn_rl_repo")

from contextlib import ExitStack

import ml_dtypes
import numpy as np

import concourse.bass as bass
import concourse.tile as tile
from concourse import bacc, mybir
from concourse.bass_utils import run_bass_kernel_spmd
from concourse.masks import make_identity

F32 = mybir.dt.float32
BF16 = mybir.dt.bfloat16
I16 = mybir.dt.int16
AF = mybir.ActivationFunctionType

N_CORES = 8
P = 128
NODE_TILE = 512   # nodes per phase-A tile (4 groups of 128)
SPLIT = 32768     # int16 index limit for dma_gather


def _table_row(c):
    """Column id -> row in the DRAM y-table (phase-A write order)."""
    c = np.asarray(c)
    t = c // NODE_TILE
    rem = c % NODE_TILE
    g = rem // P
    p = rem % P
    return (t * NODE_TILE + p * (NODE_TILE // P) + g).astype(np.int64)


def _pack_idxs(arr):
    """[128, K] slot-layout values -> dma_gather idx tile [128, 8*K] int16.

    Position i = slot*128 + p must live at idx[i%16, i//16], replicated
    across the 8 vertical 16-partition groups.
    """
    p128, K = arr.shape
    assert p128 == P
    if K == 0:
        return np.zeros((P, 0), np.int16)
    w = arr.reshape(8, 16, K).transpose(1, 2, 0).reshape(16, 8 * K)
    return np.tile(w, (8, 1)).astype(np.int16)


def _host_prep(x, edge_index, W_i, b_i, W_c, W_h, b_h, W_o, b_o):
    """Returns (in_maps, meta) for run_bass_kernel_spmd."""
    n = x.shape[0]
    npad = -(-(n + 160) // 1024) * 1024
    pad_lo = (npad - n) // 2
    nblkg = npad // P          # global 128-dst blocks
    nblk = nblkg // N_CORES    # blocks per core
    n_tiles = npad // NODE_TILE
    if npad > SPLIT:
        # hi pads (last pad columns) must land in the hi table range
        assert npad - (npad - n - pad_lo) >= SPLIT + 0 and npad >= SPLIT + 512

    row = np.concatenate([edge_index[0], np.arange(n)]).astype(np.int64)
    col = np.concatenate([edge_index[1], np.arange(n)]).astype(np.int64)

    outdeg = np.bincount(row, minlength=n).astype(np.float64)
    dis = (outdeg ** -0.5).astype(np.float32)
    dis_pad = np.zeros(npad, np.float32)   # dst-id space
    dis_pad[:n] = dis

    src_trow = _table_row(row + pad_lo)    # y-table row of each edge's source
    is_hi = src_trow >= SPLIT
    nlo = np.bincount(col[~is_hi], minlength=n)
    nhi = np.bincount(col[is_hi], minlength=n)
    nlo_pad = np.zeros(npad, np.int64)
    nlo_pad[:n] = nlo
    nhi_pad = np.zeros(npad, np.int64)
    nhi_pad[:n] = nhi

    # destination order: lex by (lo-degree desc, hi-degree desc)
    order = np.lexsort((-nhi_pad, -nlo_pad))
    pos = np.empty(npad, np.int64)
    pos[order] = np.arange(npad)
    dst_gp = order.reshape(nblkg, P)       # [global block, partition] -> dst

    # per-edge slot within (dst, lo/hi): sort by col with lo edges first
    e_order = np.lexsort((is_hi.astype(np.int8), col))
    colg = col[e_order]
    trowg = src_trow[e_order]
    ishig = is_hi[e_order]
    starts = np.searchsorted(colg, np.arange(n))
    within = np.arange(len(colg)) - starts[colg]
    slot = np.where(ishig, within - nlo[colg], within)

    # slot-count schedules, uniform across cores
    klo_g = nlo_pad[order].reshape(nblkg, P).max(1)
    khi_g = nhi_pad[order].reshape(nblkg, P).max(1)
    klo = klo_g.reshape(nblk, N_CORES).max(1).astype(np.int64)
    khi = khi_g.reshape(nblk, N_CORES).max(1).astype(np.int64)
    baseL = np.concatenate([[0], np.cumsum(klo)])
    baseH = np.concatenate([[0], np.cumsum(khi)])
    SL, SH = int(baseL[-1]), int(baseH[-1])

    # slot-layout value arrays, pads pointed at all-zero pad-node rows
    pad_hi_cnt = npad - n - pad_lo
    lo_pad_rows = _table_row(np.arange(pad_lo))
    vlo = np.empty((N_CORES, P, max(SL, 1)), np.int64)
    vlo[:, :, :] = lo_pad_rows[np.arange(P * max(SL, 1)) % pad_lo].reshape(
        P, max(SL, 1))[None]
    if SH > 0:
        hi_pad_rows = _table_row(npad - pad_hi_cnt + np.arange(pad_hi_cnt)) - SPLIT
        assert (hi_pad_rows >= 0).all()
        vhi = np.empty((N_CORES, P, SH), np.int64)
        vhi[:, :, :] = hi_pad_rows[np.arange(P * SH) % pad_hi_cnt].reshape(
            P, SH)[None]
    else:
        vhi = np.zeros((N_CORES, P, 0), np.int64)

    pe = pos[colg]
    p_e = pe % P
    g_e = pe // P
    c_e = g_e % N_CORES
    j_e = g_e // N_CORES
    lo_m = ~ishig
    vlo[c_e[lo_m], p_e[lo_m], baseL[j_e[lo_m]] + slot[lo_m]] = trowg[lo_m]
    hi_m = ishig
    vhi[c_e[hi_m], p_e[hi_m], baseH[j_e[hi_m]] + slot[hi_m]] = trowg[hi_m] - SPLIT
    assert vlo.max() < SPLIT and (SH == 0 or vhi.max() < SPLIT)

    # pack to dma_gather idx layout, concatenated per block along free dim
    def pack_core(v, k_sched, base):
        parts = [
            _pack_idxs(v[:, int(base[j]): int(base[j]) + int(k_sched[j])])
            for j in range(nblk)
        ]
        return np.concatenate(parts, axis=1) if parts else np.zeros((P, 0), np.int16)

    idxs_lo = np.stack([pack_core(vlo[c][:, :SL], klo, baseL) for c in range(N_CORES)])
    idxs_hi = np.stack([pack_core(vhi[c], khi, baseH) for c in range(N_CORES)])

    def _pad_w(a, w):
        # empty/narrow inputs become HLO constants, which bass_jit rejects
        if a.shape[2] >= w:
            return a
        out = np.zeros((a.shape[0], a.shape[1], w), np.int16)
        out[:, :, : a.shape[2]] = a
        return out

    idxs_lo = _pad_w(idxs_lo, 16)
    idxs_hi = _pad_w(idxs_hi, 16)

    # phase-A per-column scale, laid out [p, t*4+g] to match mm2 groups
    disx = np.zeros(npad, np.float32)      # column space
    disx[pad_lo: pad_lo + n] = dis
    cc = np.arange(npad)
    disA = np.zeros((P, npad // P), np.float32)
    disA[cc % P, (cc // NODE_TILE) * (NODE_TILE // P) + (cc % NODE_TILE) // P] = (
        disx[cc]
    )

    # phase-B per-dst scale, per core [p, j]
    disB_all = dis_pad[dst_gp]             # [nblkg, P]
    disB = np.stack([disB_all[c::N_CORES].T for c in range(N_CORES)])
    disB = np.ascontiguousarray(disB, dtype=np.float32)

    # host-pretransposed, padded, bf16 x (real nodes at columns pad_lo..)
    xT = np.zeros((P, npad), ml_dtypes.bfloat16)
    xT[:, pad_lo: pad_lo + n] = np.ascontiguousarray(x.T).astype(ml_dtypes.bfloat16)

    shared = {
        "xT": xT,
        "W_i": W_i.astype(ml_dtypes.bfloat16),
        "W_c": W_c.astype(ml_dtypes.bfloat16),
        "W_h": W_h.astype(np.float32),
        "W_o": W_o.astype(np.float32),
        "b_i": b_i.astype(np.float32).reshape(-1, 1),
        "b_h": b_h.astype(np.float32).reshape(-1, 1),
        "disA": disA,
    }
    in_maps = [
        {**shared, "idxs_lo": idxs_lo[c], "idxs_hi": idxs_hi[c], "disB": disB[c]}
        for c in range(N_CORES)
    ]

    meta = {
        "n": n,
        "npad": npad,
        "nblk": nblk,
        "n_tiles": n_tiles,
        "klo": klo,
        "khi": khi,
        "baseL": baseL,
        "baseH": baseH,
        "SL": SL,
        "SH": SH,
        "dst_gp": dst_gp,
        "b_o": float(np.asarray(b_o).reshape(-1)[0]),
        "in_dim": x.shape[1],
        "hid": W_i.shape[1],
        "debug_ytab": False,
    }
    return in_maps, meta


def _build(meta):
    npad = meta["npad"]
    nblk = meta["nblk"]
    n_tiles = meta["n_tiles"]
    klo, khi = meta["klo"], meta["khi"]
    baseL, baseH = meta["baseL"], meta["baseH"]
    SL, SH = meta["SL"], meta["SH"]
    in_dim = meta["in_dim"]
    hid = meta["hid"]
    grp = NODE_TILE // P  # mm2 groups per phase-A tile

    nc = bacc.Bacc()
    xT = nc.declare_dram_parameter("xT", [in_dim, npad], BF16, isOutput=False)
    W_i = nc.declare_dram_parameter("W_i", [in_dim, hid], BF16, isOutput=False)
    W_c = nc.declare_dram_parameter("W_c", [hid, hid], BF16, isOutput=False)
    W_h = nc.declare_dram_parameter("W_h", [hid, hid], F32, isOutput=False)
    W_o = nc.declare_dram_parameter("W_o", [hid, 1], F32, isOutput=False)
    b_i = nc.declare_dram_parameter("b_i", [hid, 1], F32, isOutput=False)
    b_h = nc.declare_dram_parameter("b_h", [hid, 1], F32, isOutput=False)
    disA = nc.declare_dram_parameter("disA", [P, npad // P], F32, isOutput=False)
    disB = nc.declare_dram_parameter("disB", [P, nblk], F32, isOutput=False)
    dil = nc.declare_dram_parameter("idxs_lo", [P, max(8 * SL, 16)], I16,
                                    isOutput=False)
    dih = nc.declare_dram_parameter("idxs_hi", [P, max(8 * SH, 16)], I16,
                                    isOutput=False)
    out = nc.declare_dram_parameter("out", [1, nblk * P], F32, isOutput=True)

    if meta["debug_ytab"]:
        ytab = nc.declare_dram_parameter("ytab", [npad, hid], F32, isOutput=True)
    else:
        ytab = nc.dram_tensor("ytab", [npad, hid], F32)

    with tile.TileContext(nc) as tc, ExitStack() as ctx:
        singles = ctx.enter_context(tc.tile_pool(name="singles", bufs=1))
        sWi = singles.tile([in_dim, hid], BF16)
        sWc = singles.tile([hid, hid], BF16)
        sWh = singles.tile([hid, hid], F32)
        sWo = singles.tile([hid, 1], F32)
        sbi = singles.tile([hid, 1], F32)
        sbh = singles.tile([hid, 1], F32)
        sdisA = singles.tile([P, npad // P], F32)
        sdisB = singles.tile([P, nblk], F32)
        sil = singles.tile([P, max(8 * SL, 16)], I16)
        sih = singles.tile([P, max(8 * SH, 16)], I16)
        ident = singles.tile([P, P], F32)
        outrow = singles.tile([1, nblk * P], F32)
        loads = [
            (sWi, W_i), (sWc, W_c), (sWh, W_h), (sWo, W_o),
            (sbi, b_i), (sbh, b_h), (sdisA, disA), (sdisB, disB),
        ]
        if SL > 0:
            loads.append((sil, dil))
        if SH > 0:
            loads.append((sih, dih))
        for dst_t, src_t in loads:
            nc.sync.dma_start(out=dst_t[:], in_=src_t[:])
        make_identity(nc, ident[:])

        # ---- Phase A: y table ----
        with (
            tc.tile_pool(name="pa_x", bufs=3) as pax,
            tc.tile_pool(name="pa_ps1", bufs=2, space="PSUM") as ps1,
            tc.tile_pool(name="pa_h", bufs=3) as pah,
            tc.tile_pool(name="pa_ps2", bufs=4, space="PSUM") as ps2,
            tc.tile_pool(name="pa_y", bufs=3) as pay,
        ):
            for t in range(n_tiles if not meta.get("skip_phaseA", False) else 0):
                xt = pax.tile([in_dim, NODE_TILE], BF16)
                nc.sync.dma_start(
                    out=xt[:], in_=xT[:, t * NODE_TILE:(t + 1) * NODE_TILE]
                )
                hps = ps1.tile([hid, NODE_TILE], F32)
                nc.tensor.matmul(hps[:], lhsT=sWi[:], rhs=xt[:], start=True, stop=True)
                ht = pah.tile([hid, NODE_TILE], BF16)
                nc.scalar.activation(ht[:], hps[:], AF.Relu, bias=sbi[:])
                yst = pay.tile([P, grp, hid], F32)
                for g in range(grp):
                    yps = ps2.tile([P, hid], F32)
                    nc.tensor.matmul(
                        yps[:],
                        lhsT=ht[:, g * P:(g + 1) * P],
                        rhs=sWc[:],
                        start=True,
                        stop=True,
                    )
                    nc.scalar.activation(
                        yst[:, g, :], yps[:], AF.Copy,
                        scale=sdisA[:, t * grp + g: t * grp + g + 1],
                    )
                nc.sync.dma_start(
                    out=ytab[t * NODE_TILE:(t + 1) * NODE_TILE, :].rearrange(
                        "(p g) d -> p g d", p=P
                    ),
                    in_=yst[:],
                )

        # ---- Phase B: gather + segment reduce + output head ----
        skip_b = meta.get("skip_phaseB", False)
        def reduce_slots(G, k):
            while k > 1:
                k2 = k // 2
                h = k - k2
                nc.vector.tensor_add(G[:, :k2, :], G[:, :k2, :], G[:, h:h + k2, :])
                k = h

        with (
            tc.tile_pool(name="pb_gl", bufs=3) as pbgl,
            tc.tile_pool(name="pb_gh", bufs=3) as pbgh,
            tc.tile_pool(name="pb_z", bufs=3) as pbz,
            tc.tile_pool(name="pb_pst", bufs=2, space="PSUM") as pbt,
            tc.tile_pool(name="pb_r", bufs=4) as pbr,
            tc.tile_pool(name="pb_ps2", bufs=2, space="PSUM") as pb2,
            tc.tile_pool(name="pb_pso", bufs=2, space="PSUM") as pbo,
        ):
            for j in range(nblk if not skip_b else 0):
                KL, KH = int(klo[j]), int(khi[j])
                GL = GH = None
                if KL > 0:
                    GL = pbgl.tile([P, KL, hid], F32, tag="gatherlo")
                    nc.gpsimd.dma_gather(
                        out_ap=GL[:],
                        in_ap=ytab[0:min(SPLIT, npad), :],
                        idxs_ap=sil[:, 8 * int(baseL[j]): 8 * (int(baseL[j]) + KL)],
                        num_idxs=P * KL,
                        num_idxs_reg=P * KL,
                        elem_size=hid,
                        single_packet=(P * KL <= 1024),
                    )
                    if not meta.get("phaseB_noReduce", False):
                        reduce_slots(GL, KL)
                if KH > 0:
                    GH = pbgh.tile([P, KH, hid], F32, tag="gatherhi")
                    nc.gpsimd.dma_gather(
                        out_ap=GH[:],
                        in_ap=ytab[SPLIT:npad, :],
                        idxs_ap=sih[:, 8 * int(baseH[j]): 8 * (int(baseH[j]) + KH)],
                        num_idxs=P * KH,
                        num_idxs_reg=P * KH,
                        elem_size=hid,
                        single_packet=(P * KH <= 1024),
                    )
                    if not meta.get("phaseB_noReduce", False):
                        reduce_slots(GH, KH)
                zs = pbz.tile([P, hid], F32)
                if GL is not None and GH is not None:
                    zsum = pbz.tile([P, hid], F32, tag="zsum")
                    nc.vector.tensor_add(zsum[:], GL[:, 0, :], GH[:, 0, :])
                elif GL is not None:
                    zsum = GL[:, 0, :]
                elif GH is not None:
                    zsum = GH[:, 0, :]
                else:
                    zsum = pbz.tile([P, hid], F32, tag="zsum")
                    nc.vector.memset(zsum[:], 0.0)
                nc.scalar.activation(
                    zs[:], zsum[:], AF.Copy, scale=sdisB[:, j:j + 1]
                )
                if meta.get("phaseB_noTail", False):
                    nc.vector.tensor_copy(
                        outrow[:, j * P: j * P + hid], zs[0:1, :]
                    )
                    continue
                pt = pbt.tile([hid, P], F32)
                nc.tensor.transpose(pt[:], zs[:], ident[:])
                rt = pbr.tile([hid, P], F32)
                nc.vector.tensor_copy(rt[:], pt[:])
                h2ps = pb2.tile([hid, P], F32)
                nc.tensor.matmul(h2ps[:], lhsT=sWh[:], rhs=rt[:], start=True, stop=True)
                h2 = pbr.tile([hid, P], F32, tag="h2")
                nc.scalar.activation(h2[:], h2ps[:], AF.Relu, bias=sbh[:])
                ops = pbo.tile([1, P], F32)
                nc.tensor.matmul(ops[:], lhsT=sWo[:], rhs=h2[:], start=True, stop=True)
                nc.scalar.activation(
                    outrow[:, j * P:(j + 1) * P], ops[:], AF.Identity,
                    bias=float(meta["b_o"]),
                )
        if skip_b:
            nc.vector.memset(outrow[:], 0.0)
        nc.sync.dma_start(out=out[:], in_=outrow[:])

    nc.finalize()
    return nc


def _assemble(results, meta):
    n = meta["n"]
    npad = meta["npad"]
    nblk = meta["nblk"]
    dst_gp = meta["dst_gp"]
    out_full = np.zeros(npad, np.float32)
    for c in range(N_CORES):
        vals = np.asarray(results[c]["out"]).reshape(nblk * P)
        out_full[dst_gp[c::N_CORES].ravel()] = vals
    return out_full[:n].reshape(n, 1).astype(np.float32)


def kernel(x, edge_index, W_i, b_i, W_c, W_h, b_h, W_o, b_o):
    x = np.asarray(x)
    edge_index = np.asarray(edge_index)
    in_maps, meta = _host_prep(
        x, edge_index,
        np.asarray(W_i), np.asarray(b_i), np.asarray(W_c),
        np.asarray(W_h), np.asarray(b_h), np.asarray(W_o), np.asarray(b_o),
    )
    nc = _build(meta)
    res = run_bass_kernel_spmd(nc, in_maps, list(range(N_CORES)))
    return _assemble(res.results, meta)
